# revision 16
# baseline (speedup 1.0000x reference)
"""Trainium2 Bass kernel for nn_BoundaryLoss (8-core data-parallel).

v5: d^2-1 is decoded on DVE directly from the f32 bit pattern of the
pass-2 PSUM (bitcast i32; round((K-1) - s*bits) gives exact integer
d^2 - 1; the pixel's own class decodes to -1 = 0xFFFF). Second-min of
the four planes is then just 3 unsigned-min TT ops (own class becomes
the u16 max), and both Sqrt activations re-add the +1 via their bias.
Scalar runs only Copy evac + Sqrt (one act table set). Masks:
i32->i16 cast on GpSimd, is_equal on DVE.
"""

import ml_dtypes
import numpy as np

import concourse.bacc as bacc
import concourse.bass as bass
import concourse.mybir as mybir
from concourse.mybir import AluOpType as Op
from concourse.tile import TileContext

P = 128
H = W = 512
YB = H // P          # 4 y-blocks
C = 4                # classes
BPC = 2              # images per core
NCORES = 8
B_TOTAL = BPC * NCORES

BETA = 5.0
B2 = 20.0            # exp bias, conv-Y weights
B3 = 20.0            # exp bias, conv-X weights
RND = 0.35           # rounding bias, folded into the Ln input scale

# band column ranges per 128-block: [lo_j, hi_j), even (8B-aligned) offsets;
# the widened columns carry exactly-zero bf16 weights
_RANGES = [(max(0, 128 * j - 6), min(W, 128 * (j + 1) + 6)) for j in range(YB)]
_NJ = [hi - lo for lo, hi in _RANGES]
_OFF = [sum(_NJ[:j]) for j in range(YB)]
WCOLS = sum(_NJ)

F32 = mybir.dt.float32
BF16 = mybir.dt.bfloat16
I32 = mybir.dt.int32
I16 = mybir.dt.int16
U16 = mybir.dt.uint16
Act = mybir.ActivationFunctionType

LNSCALE = float(np.exp(np.float64(-(B2 + B3) - BETA * RND)))
# f32-bit decode: v = round(K32 - S32 * bitcast_i32(x))
S32 = float(np.log(2.0) / (BETA * 2.0 ** 23))
RNDC = 0.26
K32 = float((B2 + B3) / BETA + 127.0 * np.log(2.0) / BETA + RNDC)


def _build_nc():
    nc = bacc.Bacc("TRN2", target_bir_lowering=False, debug=False)
    tgt_d = nc.dram_tensor("target", [BPC, H, W], I32, kind="ExternalInput")
    wm_d = nc.dram_tensor("wmats", [P, WCOLS], BF16, kind="ExternalInput")
    osp_d = nc.dram_tensor("osum_p", [P, BPC * YB + 2], F32,
                           kind="ExternalOutput")
    osm_d = nc.dram_tensor("osum_m", [P, BPC * YB + 2], F32,
                           kind="ExternalOutput")

    with TileContext(nc) as tc:
        with (
            tc.tile_pool(name="const", bufs=1) as cpool,
            tc.tile_pool(name="tgt", bufs=5) as tgt_pool,
            tc.tile_pool(name="mask", bufs=2 * YB) as m_pool,
            tc.tile_pool(name="stp", bufs=2 * YB) as st_pool,
            tc.tile_pool(name="d2r", bufs=4) as d2_pool,
            tc.tile_pool(name="scratch", bufs=8) as s_pool,
            tc.tile_pool(name="acc", bufs=1) as a_pool,
            tc.tile_pool(name="ps", bufs=2, space="PSUM") as ps_pool,
        ):
            bias0 = cpool.tile([P, 1], F32)
            nc.vector.memset(bias0, 0.0)
            bias1 = cpool.tile([P, 1], F32)
            nc.vector.memset(bias1, 1.0)
            wtile = cpool.tile([P, WCOLS], BF16)
            nc.sync.dma_start(wtile, wm_d[:, :])
            wband = [wtile[:, _OFF[j]:_OFF[j] + _NJ[j]] for j in range(YB)]

            accp = a_pool.tile([P, BPC * YB + 2], F32)
            accm = a_pool.tile([P, BPC * YB + 2], F32)

            # ---- masks for both images up front: i32->i16 cast on
            #      GpSimd, (t==c) on DVE ----
            m_tiles_b = [[], []]
            for b in range(BPC):
                for yb in range(YB):
                    tgt = tgt_pool.tile([P, W], I32)
                    nc.sync.dma_start(tgt, tgt_d[b, yb * P:(yb + 1) * P, :])
                    t16 = tgt_pool.tile([P, W], I16, tag="t16")
                    if b == 0:
                        nc.vector.tensor_copy(t16, tgt)
                    else:
                        nc.gpsimd.tensor_copy(t16, tgt)
                    mw = m_pool.tile([P, C, W], BF16)
                    for c in range(C):
                        nc.vector.tensor_scalar(mw[:, c], t16, c, None,
                                                Op.is_equal)
                    m_tiles_b[b].append(mw)

            st_tiles_b = [[], []]

            def pass1_unit(b, xb):
                m_tiles = m_tiles_b[b]
                ps = ps_pool.tile([P, C, W], F32)
                for c in range(C):
                    for j in range(YB):
                        lo, hi = _RANGES[j]
                        nc.tensor.matmul(
                            ps[:, c, lo:hi],
                            m_tiles[j][:, c, xb * P:(xb + 1) * P],
                            wband[j],
                            start=(j == 0), stop=(j == YB - 1),
                        )
                st = st_pool.tile([P, C, W], BF16)
                nc.scalar.activation(st[:], ps[:], Act.Copy)
                st_tiles_b[b].append(st)

            def decode_unit(ps, wcols, lo, colp, colx):
                # decode d^2 - 1 straight from f32 bits (no Ln): 1 DVE op;
                # own-class pixels decode to -1 (0xFFFF as u16)
                d2 = d2_pool.tile([P, C, wcols], I16, tag="d2")
                nc.vector.tensor_scalar(d2[:], ps[:, :, lo:lo + wcols]
                                        .bitcast(I32),
                                        -S32, K32 - 1.0, Op.mult, Op.add)
                # accp: sum of sqrt((d^2-1) + 1); own class -> sqrt(0)
                dd = s_pool.tile([P, C, wcols], BF16, tag="dd")
                nc.scalar.activation(
                    dd[:], d2[:], Act.Sqrt,
                    bias=bias1[:], accum_out=accp[:, colp:colp + 1],
                )
                # secondmin-1 = unsigned min over the 4 planes
                # (own class is 0xFFFF = u16 max, so it never wins)
                mn2 = s_pool.tile([P, 2, wcols], U16, tag="mn2")
                nc.vector.tensor_tensor(mn2[:, 0], d2[:, 0].bitcast(U16),
                                        d2[:, 1].bitcast(U16), Op.min)
                nc.vector.tensor_tensor(mn2[:, 1], d2[:, 2].bitcast(U16),
                                        d2[:, 3].bitcast(U16), Op.min)
                sm = s_pool.tile([P, wcols], U16, tag="sm")
                nc.vector.tensor_tensor(sm[:], mn2[:, 0], mn2[:, 1], Op.min)
                # accm: sum of sqrt((secondmin-1) + 1)
                mscr = s_pool.tile([P, wcols], BF16, tag="mscr")
                nc.scalar.activation(
                    mscr[:], sm[:], Act.Sqrt,
                    bias=bias1[:], accum_out=accm[:, colx:colx + 1],
                )

            def pass2_unit(b, yb, split=False):
                st_tiles = st_tiles_b[b]
                ps = ps_pool.tile([P, C, W], F32)
                for c in range(C):
                    for k in range(YB):
                        lo, hi = _RANGES[k]
                        nc.tensor.matmul(
                            ps[:, c, lo:hi],
                            st_tiles[k][:, c, yb * P:(yb + 1) * P],
                            wband[k],
                            start=(k == 0), stop=(k == YB - 1),
                        )
                colp = b * YB + yb
                if not split:
                    decode_unit(ps, W, 0, colp, colp)
                else:
                    # halve the drain chain of the last units: second half
                    # accumulates into a spare column (host sums them all)
                    xcol = BPC * YB + (yb - 2)
                    decode_unit(ps, W // 2, 0, colp, colp)
                    decode_unit(ps, W // 2, W // 2, xcol, xcol)

            # pass1 image-major (b0's st tiles complete first, so pass2(b0)
            # overlaps pass1(b1) on the PE); pass2 image-major
            for b in range(BPC):
                for xb in range(YB):
                    pass1_unit(b, xb)
            for b in range(BPC):
                for yb in range(YB):
                    pass2_unit(b, yb, split=(b == 1 and yb >= 2))

            nc.sync.dma_start(osp_d[:, :], accp[:])
            nc.sync.dma_start(osm_d[:, :], accm[:])

    nc.compile()
    return nc


_NC = None
_WM = None


def _host_wmats():
    """Band-sliced Gaussian weight tiles, bf16, packed [P, WCOLS]."""
    global _WM
    if _WM is None:
        k = np.arange(P)[:, None].astype(np.float32)
        cols = []
        for j in range(YB):
            lo, hi = _RANGES[j]
            m = np.arange(lo, hi)[None, :].astype(np.float32)
            idx = 128 * j + k - m
            cols.append(np.exp(np.float32(B2) - np.float32(BETA) * idx * idx,
                               dtype=np.float32))
        _WM = np.concatenate(cols, axis=1).astype(ml_dtypes.bfloat16)
    return _WM


def _get_nc():
    global _NC
    if _NC is None:
        _NC = _build_nc()
    return _NC


def _exact_fallback(pred, target):
    """Exact numpy implementation of the reference (slow; adversarial inputs only)."""
    THETA0, THETA, RR = 3.0, 5.0, 5
    offs = [(dy, dx, float(np.hypot(dy, dx)))
            for dy in range(-RR, RR + 1) for dx in range(-RR, RR + 1)
            if np.hypot(dy, dx) <= THETA]

    def capped_edt(ts):
        B, Hh, Ww = ts.shape
        pad = np.zeros((B, Hh + 2 * RR, Ww + 2 * RR), bool)
        pad[:, RR:-RR, RR:-RR] = ts
        d = np.full((B, Hh, Ww), THETA, np.float32)
        for dy, dx, dist in offs:
            win = pad[:, RR + dy:RR + dy + Hh, RR + dx:RR + dx + Ww]
            d = np.minimum(d, np.where(win, np.float32(dist), np.float32(THETA)))
        return d

    def compute_sdf(mask):
        sdf_pos = capped_edt(mask == 1.0)
        sdf_neg = capped_edt(mask == 0.0)
        sdf = np.clip(sdf_pos - sdf_neg, -THETA, THETA) / THETA
        empty = mask.sum(axis=(1, 2)) == 0.0
        return np.where(empty[:, None, None], np.float32(THETA0), sdf).astype(np.float32)

    x = pred.astype(np.float32)
    x = x - x.max(axis=1, keepdims=True)
    ex = np.exp(x)
    p = ex / ex.sum(axis=1, keepdims=True)
    Cn = pred.shape[1]
    loss = np.float32(0.0)
    for c in range(Cn):
        ps = compute_sdf(p[:, c].astype(np.float32))
        ts = compute_sdf((target == c).astype(np.float32))
        loss += np.abs(ps - ts).mean(dtype=np.float32)
    return np.float32(loss / Cn)


def kernel(pred: np.ndarray, target: np.ndarray) -> np.ndarray:
    pred = np.asarray(pred)
    target = np.asarray(target)

    gap_ok = float(pred.max()) - float(pred.min()) < 15.0
    tgt_ok = bool(((target >= 0) & (target < C)).all())
    present = np.array([[(target[b] == c).any() for c in range(C)]
                        for b in range(B_TOTAL)])
    if not (gap_ok and tgt_ok and present.all()):
        return _exact_fallback(pred, target)

    from concourse.bass_utils import run_bass_kernel_spmd

    nc = _get_nc()
    wm = _host_wmats()
    in_maps = [
        {"target": np.ascontiguousarray(target[i * BPC:(i + 1) * BPC]),
         "wmats": wm}
        for i in range(NCORES)
    ]
    try:
        res = run_bass_kernel_spmd(nc, in_maps, list(range(NCORES))).results
    except Exception:
        import time as _time
        _time.sleep(3.0)
        res = run_bass_kernel_spmd(nc, in_maps, list(range(NCORES))).results

    npx = H * W
    total = 0.0
    for core in range(NCORES):
        total += float(res[core]["osum_p"].astype(np.float64).sum())
        total += float(res[core]["osum_m"].astype(np.float64).sum())
    loss = total / (5.0 * npx * B_TOTAL * C)
    return np.float32(loss)


# revision 17
# speedup vs baseline: 1.0375x; 1.0375x over previous
"""Trainium2 Bass kernel for nn_BoundaryLoss (8-core data-parallel).

v5: d^2-1 is decoded on DVE directly from the f32 bit pattern of the
pass-2 PSUM (bitcast i32; round((K-1) - s*bits) gives exact integer
d^2 - 1; the pixel's own class decodes to -1 = 0xFFFF). Second-min of
the four planes is then just 3 unsigned-min TT ops (own class becomes
the u16 max), and both Sqrt activations re-add the +1 via their bias.
Scalar runs only Copy evac + Sqrt (one act table set). Masks:
i32->i16 cast on GpSimd, is_equal on DVE.
"""

import ml_dtypes
import numpy as np

import concourse.bacc as bacc
import concourse.bass as bass
import concourse.mybir as mybir
from concourse.mybir import AluOpType as Op
from concourse.tile import TileContext

P = 128
H = W = 512
YB = H // P          # 4 y-blocks
C = 4                # classes
BPC = 2              # images per core
NCORES = 8
B_TOTAL = BPC * NCORES

BETA = 5.0
B2 = 20.0            # exp bias, conv-Y weights
B3 = 20.0            # exp bias, conv-X weights
RND = 0.35           # rounding bias, folded into the Ln input scale

# band column ranges per 128-block: [lo_j, hi_j), even (8B-aligned) offsets;
# the widened columns carry exactly-zero bf16 weights
_RANGES = [(max(0, 128 * j - 6), min(W, 128 * (j + 1) + 6)) for j in range(YB)]
_NJ = [hi - lo for lo, hi in _RANGES]
_OFF = [sum(_NJ[:j]) for j in range(YB)]
WCOLS = sum(_NJ)

F32 = mybir.dt.float32
BF16 = mybir.dt.bfloat16
I32 = mybir.dt.int32
I16 = mybir.dt.int16
U16 = mybir.dt.uint16
Act = mybir.ActivationFunctionType

LNSCALE = float(np.exp(np.float64(-(B2 + B3) - BETA * RND)))
# f32-bit decode: v = round(K32 - S32 * bitcast_i32(x))
S32 = float(np.log(2.0) / (BETA * 2.0 ** 23))
RNDC = 0.26
K32 = float((B2 + B3) / BETA + 127.0 * np.log(2.0) / BETA + RNDC)


def _build_nc():
    nc = bacc.Bacc("TRN2", target_bir_lowering=False, debug=False)
    tgt_d = nc.dram_tensor("target", [BPC, H, W], I32, kind="ExternalInput")
    wm_d = nc.dram_tensor("wmats", [P, WCOLS], BF16, kind="ExternalInput")
    osp_d = nc.dram_tensor("osum_p", [P, BPC * YB], F32, kind="ExternalOutput")
    osm_d = nc.dram_tensor("osum_m", [P, BPC * YB], F32, kind="ExternalOutput")

    with TileContext(nc) as tc:
        with (
            tc.tile_pool(name="const", bufs=1) as cpool,
            tc.tile_pool(name="tgt", bufs=5) as tgt_pool,
            tc.tile_pool(name="mask", bufs=2 * YB) as m_pool,
            tc.tile_pool(name="stp", bufs=2 * YB) as st_pool,
            tc.tile_pool(name="d2r", bufs=4) as d2_pool,
            tc.tile_pool(name="scratch", bufs=8) as s_pool,
            tc.tile_pool(name="acc", bufs=1) as a_pool,
            tc.tile_pool(name="ps", bufs=2, space="PSUM") as ps_pool,
        ):
            bias0 = cpool.tile([P, 1], F32)
            nc.vector.memset(bias0, 0.0)
            bias1 = cpool.tile([P, 1], F32)
            nc.vector.memset(bias1, 1.0)
            wtile = cpool.tile([P, WCOLS], BF16)
            nc.sync.dma_start(wtile, wm_d[:, :])
            wband = [wtile[:, _OFF[j]:_OFF[j] + _NJ[j]] for j in range(YB)]

            accp = a_pool.tile([P, BPC * YB], F32)
            accm = a_pool.tile([P, BPC * YB], F32)

            # ---- masks for both images up front: i32->i16 cast on
            #      GpSimd, (t==c) on DVE ----
            m_tiles_b = [[], []]
            for b in range(BPC):
                for yb in range(YB):
                    tgt = tgt_pool.tile([P, W], I32)
                    nc.sync.dma_start(tgt, tgt_d[b, yb * P:(yb + 1) * P, :])
                    t16 = tgt_pool.tile([P, W], I16, tag="t16")
                    if b == 0:
                        nc.vector.tensor_copy(t16, tgt)
                    else:
                        nc.gpsimd.tensor_copy(t16, tgt)
                    mw = m_pool.tile([P, C, W], BF16)
                    for c in range(C):
                        nc.vector.tensor_scalar(mw[:, c], t16, c, None,
                                                Op.is_equal)
                    m_tiles_b[b].append(mw)

            st_tiles_b = [[], []]

            def pass1_unit(b, xb):
                m_tiles = m_tiles_b[b]
                ps = ps_pool.tile([P, C, W], F32)
                for c in range(C):
                    for j in range(YB):
                        lo, hi = _RANGES[j]
                        nc.tensor.matmul(
                            ps[:, c, lo:hi],
                            m_tiles[j][:, c, xb * P:(xb + 1) * P],
                            wband[j],
                            start=(j == 0), stop=(j == YB - 1),
                        )
                st = st_pool.tile([P, C, W], BF16)
                nc.scalar.activation(st[:], ps[:], Act.Copy)
                st_tiles_b[b].append(st)

            def pass2_unit(b, yb):
                st_tiles = st_tiles_b[b]
                ps = ps_pool.tile([P, C, W], F32)
                for c in range(C):
                    for k in range(YB):
                        lo, hi = _RANGES[k]
                        nc.tensor.matmul(
                            ps[:, c, lo:hi],
                            st_tiles[k][:, c, yb * P:(yb + 1) * P],
                            wband[k],
                            start=(k == 0), stop=(k == YB - 1),
                        )
                # decode d^2 - 1 straight from f32 bits (no Ln): 1 DVE op;
                # own-class pixels decode to -1 (0xFFFF as u16)
                d2 = d2_pool.tile([P, C, W], I16)
                nc.vector.tensor_scalar(d2[:], ps[:].bitcast(I32),
                                        -S32, K32 - 1.0, Op.mult, Op.add)

                colp = b * YB + yb
                # accp: sum of sqrt((d^2-1) + 1); own class -> sqrt(0)
                dd = s_pool.tile([P, C, W], BF16, tag="dd")
                nc.scalar.activation(
                    dd[:], d2[:], Act.Sqrt,
                    bias=bias1[:], accum_out=accp[:, colp:colp + 1],
                )

                # secondmin-1 = unsigned min over the 4 planes
                # (own class is 0xFFFF = u16 max, so it never wins)
                mn2 = s_pool.tile([P, 2, W], U16, tag="mn2")
                nc.vector.tensor_tensor(mn2[:, 0], d2[:, 0].bitcast(U16),
                                        d2[:, 1].bitcast(U16), Op.min)
                nc.vector.tensor_tensor(mn2[:, 1], d2[:, 2].bitcast(U16),
                                        d2[:, 3].bitcast(U16), Op.min)
                sm = s_pool.tile([P, W], U16, tag="sm")
                nc.vector.tensor_tensor(sm[:], mn2[:, 0], mn2[:, 1], Op.min)
                # accm: sum of sqrt((secondmin-1) + 1)
                mscr = s_pool.tile([P, W], BF16, tag="mscr")
                nc.scalar.activation(
                    mscr[:], sm[:], Act.Sqrt,
                    bias=bias1[:], accum_out=accm[:, colp:colp + 1],
                )

            # pass1 image-major (b0's st tiles complete first, so pass2(b0)
            # overlaps pass1(b1) on the PE); pass2 image-major
            for b in range(BPC):
                for xb in range(YB):
                    pass1_unit(b, xb)
            for b in range(BPC):
                for yb in range(YB):
                    pass2_unit(b, yb)

            nc.sync.dma_start(osp_d[:, :], accp[:])
            nc.sync.dma_start(osm_d[:, :], accm[:])

    nc.compile()
    return nc


_NC = None
_WM = None


def _host_wmats():
    """Band-sliced Gaussian weight tiles, bf16, packed [P, WCOLS]."""
    global _WM
    if _WM is None:
        k = np.arange(P)[:, None].astype(np.float32)
        cols = []
        for j in range(YB):
            lo, hi = _RANGES[j]
            m = np.arange(lo, hi)[None, :].astype(np.float32)
            idx = 128 * j + k - m
            cols.append(np.exp(np.float32(B2) - np.float32(BETA) * idx * idx,
                               dtype=np.float32))
        _WM = np.concatenate(cols, axis=1).astype(ml_dtypes.bfloat16)
    return _WM


def _get_nc():
    global _NC
    if _NC is None:
        _NC = _build_nc()
    return _NC


def _exact_fallback(pred, target):
    """Exact numpy implementation of the reference (slow; adversarial inputs only)."""
    THETA0, THETA, RR = 3.0, 5.0, 5
    offs = [(dy, dx, float(np.hypot(dy, dx)))
            for dy in range(-RR, RR + 1) for dx in range(-RR, RR + 1)
            if np.hypot(dy, dx) <= THETA]

    def capped_edt(ts):
        B, Hh, Ww = ts.shape
        pad = np.zeros((B, Hh + 2 * RR, Ww + 2 * RR), bool)
        pad[:, RR:-RR, RR:-RR] = ts
        d = np.full((B, Hh, Ww), THETA, np.float32)
        for dy, dx, dist in offs:
            win = pad[:, RR + dy:RR + dy + Hh, RR + dx:RR + dx + Ww]
            d = np.minimum(d, np.where(win, np.float32(dist), np.float32(THETA)))
        return d

    def compute_sdf(mask):
        sdf_pos = capped_edt(mask == 1.0)
        sdf_neg = capped_edt(mask == 0.0)
        sdf = np.clip(sdf_pos - sdf_neg, -THETA, THETA) / THETA
        empty = mask.sum(axis=(1, 2)) == 0.0
        return np.where(empty[:, None, None], np.float32(THETA0), sdf).astype(np.float32)

    x = pred.astype(np.float32)
    x = x - x.max(axis=1, keepdims=True)
    ex = np.exp(x)
    p = ex / ex.sum(axis=1, keepdims=True)
    Cn = pred.shape[1]
    loss = np.float32(0.0)
    for c in range(Cn):
        ps = compute_sdf(p[:, c].astype(np.float32))
        ts = compute_sdf((target == c).astype(np.float32))
        loss += np.abs(ps - ts).mean(dtype=np.float32)
    return np.float32(loss / Cn)


def kernel(pred: np.ndarray, target: np.ndarray) -> np.ndarray:
    pred = np.asarray(pred)
    target = np.asarray(target)

    gap_ok = float(pred.max()) - float(pred.min()) < 15.0
    tgt_ok = bool(((target >= 0) & (target < C)).all())
    present = np.array([[(target[b] == c).any() for c in range(C)]
                        for b in range(B_TOTAL)])
    if not (gap_ok and tgt_ok and present.all()):
        return _exact_fallback(pred, target)

    from concourse.bass_utils import run_bass_kernel_spmd

    nc = _get_nc()
    wm = _host_wmats()
    in_maps = [
        {"target": np.ascontiguousarray(target[i * BPC:(i + 1) * BPC]),
         "wmats": wm}
        for i in range(NCORES)
    ]
    try:
        res = run_bass_kernel_spmd(nc, in_maps, list(range(NCORES))).results
    except Exception:
        import time as _time
        _time.sleep(3.0)
        res = run_bass_kernel_spmd(nc, in_maps, list(range(NCORES))).results

    npx = H * W
    total = 0.0
    for core in range(NCORES):
        total += float(res[core]["osum_p"].astype(np.float64).sum())
        total += float(res[core]["osum_m"].astype(np.float64).sum())
    loss = total / (5.0 * npx * B_TOTAL * C)
    return np.float32(loss)
